# revision 11
# baseline (speedup 1.0000x reference)
"""GAT (3-layer, PyG-style) forward on 8 Trainium2 NeuronCores via Bass/Tile.

Strategy (dst-partitioned edges + AllGathered projection table), tuned around
the hard cost on TRN2: SWDGE descriptor generation on the GpSimd Q7 runs at
~8ns per gathered row, so per-edge indirection is reserved for the one place
it is unavoidable (layer-1/2 source-row fetch) and everything else uses
sequential HWDGE streams or PE one-hot matmuls:
  - Nodes are split into 8 contiguous shards (6250 each). Each core owns the
    edges whose *destination* lies in its shard, grouped into 128-node dst
    windows; each window's edges are split by which half of the padded shard
    their source row sits in (so int16 dma_gather indices reach the whole
    AllGathered half-table), padded to uniform 128-edge tiles across cores.
  - Layer 0 needs no gather: the host pre-gathers x into edge order (xE,
    feature-major); tiles are projected on the PE (xE^T @ Waug), four tiles
    packed per PSUM bank.
  - Layers 1/2: each core projects its shard into rows
    [xp | a_src | a_dst | pad->384]; the two window-halves of the shard are
    AllGathered separately (each collective is issued as soon as the
    half-layer tail that produces it finishes, so it overlaps the rest of
    the previous layer); per 3-window chunk TWO dma_gathers (one per
    half-table) fetch all source rows, with the lo-gathers running two
    chunks ahead of the hi-gathers.
  - Per-edge a_dst comes from PE one-hot matmuls (sd[v,t,e] host-shipped,
    streamed sequentially) against the window's local rows — no descriptors.
  - S[e,t,v] (dst one-hot) is host-shipped too; the segment softmax
    scatter-add is PE matmuls accumulating numerator+denominator in PSUM,
    with the self-loop folded in as one identity-matmul per window.
  - Softmax uses exp without max subtraction (logits are O(1)).
  - One-source copies/scales and the per-head normalize run on the Scalar
    engine (Copy lives in every ACT table set); exp/gelu grouped so each
    layer pays ~2 table loads.
  - Normalized outputs land in one bf16 slab per layer; bias+GELU once per
    half-layer, then per-window PE transposes + projection build the next
    layer's table. Global mean pool via one-hot(batch) matmuls + AllReduce.
"""

import math
import numpy as np

import concourse.bass as bass
import concourse.bacc as bacc
import concourse.mybir as mybir
import concourse.tile as tile
from concourse.masks import make_identity

F32 = mybir.dt.float32
BF16 = mybir.dt.bfloat16
I16 = mybir.dt.int16

CHW = 3  # windows per gather/compute chunk


def _pad_row(r):
    return ((r + 127) // 128) * 128  # bf16 rows to 256B multiples


class GATCfg:
    def __init__(self, N, E, B, Fin, layers, NC=8):
        self.N, self.E, self.B, self.Fin, self.NC = N, E, B, Fin, NC
        assert N % NC == 0
        self.NPC = N // NC
        self.NW = math.ceil(self.NPC / 128)
        self.NPCp = self.NW * 128
        lim = (32767 // (NC * 128)) // CHW * CHW   # int16 idx bound
        self.NWH = min(lim, ((self.NW - 1) // CHW) * CHW)  # windows in half A
        self.HNPA = self.NWH * 128             # rows in half A
        self.HNPB = self.NPCp - self.HNPA      # rows in half B
        self.layers = []
        d_in = Fin
        for l in layers:
            H, C, concat = l["H"], l["C"], l["concat"]
            d_out = H * C
            self.layers.append(
                dict(d_in=d_in, H=H, C=C, d_out=d_out, concat=concat,
                     R2=d_out + H, db=(d_out if concat else C),
                     ROW=d_out + 2 * H, ROWP=_pad_row(d_out + 2 * H))
            )
            d_in = d_out if concat else C


REAL_CFG = GATCfg(
    N=50000, E=400000, B=64, Fin=128,
    layers=[dict(H=4, C=16, concat=True),
            dict(H=4, C=64, concat=True),
            dict(H=4, C=64, concat=False)],
)


def _wrap16(unwrapped):
    """dma_gather idx layout: element i -> (partition i%16, col i//16),
    replicated across the 8 groups of 16 partitions."""
    w16 = unwrapped.reshape(-1, 16).T  # [16, n/16]
    return np.tile(w16, (8, 1)).astype(np.int16)


# ---------------------------------------------------------------- host prep
def _host_prep(cfg, x, edge_index, batch, Ws, As, Ad, Bs):
    import ml_dtypes

    N, NC, NPC, NPCp, NW = cfg.N, cfg.NC, cfg.NPC, cfg.NPCp, cfg.NW
    HNPA, HNPB = cfg.HNPA, cfg.HNPB
    src = np.asarray(edge_index[0], dtype=np.int64)
    dst = np.asarray(edge_index[1], dtype=np.int64)
    core_of = dst // NPC
    score = src // NPC
    rloc = src % NPC                      # row within the owner's shard
    half = (rloc >= HNPA).astype(np.int64)
    win = (dst % NPC) // 128

    cnts = np.zeros((NC, NW, 2), np.int64)
    np.add.at(cnts, (core_of, win, half), 1)
    twl = np.ceil(cnts[:, :, 0].max(axis=0) / 128).astype(int)
    twh = np.ceil(cnts[:, :, 1].max(axis=0) / 128).astype(int)
    for w in range(NW):
        if twl[w] + twh[w] == 0:
            twl[w] = 1

    chunks = [(c0, min(c0 + CHW, NW)) for c0 in range(0, NW, CHW)]
    lo_t0 = np.zeros(NW, int)
    hi_t0 = np.zeros(NW, int)
    cinfo = []
    tile_base = lo_cum = hi_cum = 0
    for (c0, c1) in chunks:
        nlo = int(twl[c0:c1].sum())
        nhi = int(twh[c0:c1].sum())
        t = tile_base
        for w in range(c0, c1):
            lo_t0[w] = t
            t += twl[w]
        for w in range(c0, c1):
            hi_t0[w] = t
            t += twh[w]
        cinfo.append(dict(tile0=tile_base, nlo=nlo, nhi=nhi,
                          lo_base=lo_cum, hi_base=hi_cum))
        lo_cum += nlo
        hi_cum += nhi
        tile_base += nlo + nhi
    TOT, TOTLO, TOTHI = tile_base, lo_cum, hi_cum

    chunk_of = np.zeros(NW, int)
    for ci, (c0, c1) in enumerate(chunks):
        chunk_of[c0:c1] = ci

    per_core = []
    for c in range(NC):
        sel = np.nonzero(core_of == c)[0]
        dloc = (dst[sel] - c * NPC).astype(np.int64)
        sg = src[sel]
        sc = score[sel]
        rl = rloc[sel]
        hf = half[sel]
        wn = dloc // 128
        order = np.argsort(wn * 2 + hf, kind="stable")
        sel, dloc, sg, sc, rl, hf, wn = (sel[order], dloc[order], sg[order],
                                         sc[order], rl[order], hf[order], wn[order])
        gid = wn * 2 + hf
        gstart = np.searchsorted(gid, np.arange(2 * NW))
        slot = np.arange(len(sel)) - gstart[gid]
        jj, pp = slot // 128, slot % 128
        t0 = np.where(hf == 0, lo_t0[wn], hi_t0[wn])
        tcol = t0 + jj

        drel = np.full((128, TOT), -1, np.int64)
        drel[pp, tcol] = dloc - wn * 128

        Sp = (drel[:, :, None] == np.arange(128)[None, None, :])
        Sp = Sp.astype(ml_dtypes.bfloat16)
        sdp = np.ascontiguousarray(Sp.transpose(2, 1, 0))

        srcn = np.zeros((128, TOT), np.int64)
        srcn[pp, tcol] = sg
        xE = np.ascontiguousarray(
            x[srcn.T.reshape(-1)].T.reshape(cfg.Fin, TOT, 128)
        ).astype(ml_dtypes.bfloat16)

        # gather indices into the half-tables (core-major within each half)
        un_lo = np.zeros(TOTLO * 128, np.int64)
        un_hi = np.zeros(TOTHI * 128, np.int64)
        ci = chunk_of[wn]
        tile0 = np.array([d["tile0"] for d in cinfo])[ci]
        lo_base = np.array([d["lo_base"] for d in cinfo])[ci]
        hi_base = np.array([d["hi_base"] for d in cinfo])[ci]
        nlo_arr = np.array([d["nlo"] for d in cinfo])[ci]
        islo = hf == 0
        pos_lo = (lo_base[islo] + (tcol[islo] - tile0[islo])) * 128 + pp[islo]
        un_lo[pos_lo] = sc[islo] * HNPA + rl[islo]
        pos_hi = (hi_base[~islo] + (tcol[~islo] - tile0[~islo] - nlo_arr[~islo])) * 128 + pp[~islo]
        un_hi[pos_hi] = sc[~islo] * HNPB + (rl[~islo] - HNPA)

        batchw = np.full((128, NW), -1.0, np.float32)
        bf = np.full(NPCp, -1.0, np.float32)
        bf[:NPC] = batch[c * NPC:(c + 1) * NPC].astype(np.float32)
        batchw[:, :] = bf.reshape(NW, 128).T

        xT = np.zeros((cfg.Fin, NPCp), np.float32)
        xT[:, :NPC] = x[c * NPC:(c + 1) * NPC].T

        m = dict(
            xT=xT.astype(ml_dtypes.bfloat16),
            xE=xE, Sp=Sp, sdp=sdp,
            esrcL=_wrap16(un_lo), esrcH=_wrap16(un_hi),
            batchw=batchw.astype(ml_dtypes.bfloat16),
        )
        for li, (W, a_s, a_d) in enumerate(zip(Ws, As, Ad)):
            L = cfg.layers[li]
            H, C, d_in, ROWP = L["H"], L["C"], L["d_in"], L["ROWP"]
            Wr = W.reshape(d_in, H, C)
            Wts = np.einsum("khc,hc->kh", Wr, a_s).astype(np.float32)
            Wtd = np.einsum("khc,hc->kh", Wr, a_d).astype(np.float32)
            wa = np.zeros((d_in, ROWP), np.float32)
            wa[:, :L["ROW"]] = np.concatenate([W, Wts, Wtd], axis=1)
            m[f"waug{li}"] = wa.astype(ml_dtypes.bfloat16)
            m[f"bias{li}"] = np.broadcast_to(Bs[li], (128, L["db"])).astype(np.float32).copy()
        per_core.append(m)

    meta = dict(chunks=chunks, cinfo=cinfo, twl=twl, twh=twh,
                lo_t0=lo_t0, hi_t0=hi_t0, TOT=TOT, TOTLO=TOTLO, TOTHI=TOTHI)
    return per_core, meta


# ---------------------------------------------------------------- program
def _build_program(cfg, meta):
    NC, NPCp, NW, B = cfg.NC, cfg.NPCp, cfg.NW, cfg.B
    NWH, HNPA, HNPB = cfg.NWH, cfg.HNPA, cfg.HNPB
    NL = len(cfg.layers)
    chunks, cinfo = meta["chunks"], meta["cinfo"]
    twl, twh = meta["twl"], meta["twh"]
    lo_t0, hi_t0 = meta["lo_t0"], meta["hi_t0"]
    TOT, TOTLO, TOTHI = meta["TOT"], meta["TOTLO"], meta["TOTHI"]
    NCH_A = NWH // CHW          # chunks whose windows all sit in half A
    PF = 2                      # lo-gather prefetch depth (chunks)

    nc = bacc.Bacc("TRN2", target_bir_lowering=False, debug=False,
                   enable_asserts=False, num_devices=NC)

    # ---- I/O
    xT_p = nc.declare_dram_parameter("xT", [cfg.Fin, NPCp], BF16, isOutput=False)
    xE_p = nc.declare_dram_parameter("xE", [cfg.Fin, TOT, 128], BF16, isOutput=False)
    S_p = nc.declare_dram_parameter("Sp", [128, TOT, 128], BF16, isOutput=False)
    sd_p = nc.declare_dram_parameter("sdp", [128, TOT, 128], BF16, isOutput=False)
    esrcL_p = nc.declare_dram_parameter("esrcL", [128, TOTLO * 8], I16, isOutput=False)
    esrcH_p = nc.declare_dram_parameter("esrcH", [128, TOTHI * 8], I16, isOutput=False)
    batchw_p = nc.declare_dram_parameter("batchw", [128, NW], BF16, isOutput=False)
    waug_p, bias_p = [], []
    for li, L in enumerate(cfg.layers):
        waug_p.append(nc.declare_dram_parameter(f"waug{li}", [L["d_in"], L["ROWP"]], BF16, isOutput=False))
        bias_p.append(nc.declare_dram_parameter(f"bias{li}", [128, L["db"]], F32, isOutput=False))
    out_p = nc.declare_dram_parameter("out", [B, cfg.layers[-1]["C"]], F32, isOutput=True)

    # ---- internal DRAM
    tabloc = [nc.dram_tensor(f"tabloc{li}", [NPCp, L["ROWP"]], BF16)
              for li, L in enumerate(cfg.layers)]
    tabfA = [None] + [nc.dram_tensor(f"tabfA{li}", [NC * HNPA, cfg.layers[li]["ROWP"]],
                                     BF16, addr_space="Shared") for li in (1, 2)]
    tabfB = [None] + [nc.dram_tensor(f"tabfB{li}", [NC * HNPB, cfg.layers[li]["ROWP"]],
                                     BF16, addr_space="Shared") for li in (1, 2)]
    poolpart = nc.dram_tensor("poolpart", [B, cfg.layers[-1]["C"] + 1], F32)
    poolsum = nc.dram_tensor("poolsum", [B, cfg.layers[-1]["C"] + 1], F32, addr_space="Shared")

    rg = [list(range(NC))]
    Gelu = mybir.ActivationFunctionType.Gelu
    Exp = mybir.ActivationFunctionType.Exp
    Copy = mybir.ActivationFunctionType.Copy

    with tile.TileContext(nc) as tc:
        with (
            tc.tile_pool(name="const", bufs=1) as constp,
            tc.tile_pool(name="wts", bufs=1) as wtsp,
            tc.tile_pool(name="proj", bufs=3) as projp,
            tc.tile_pool(name="glo", bufs=PF + 1) as glop,
            tc.tile_pool(name="ghi", bufs=2) as ghip,
            tc.tile_pool(name="edge", bufs=2) as edgep,
            tc.tile_pool(name="lay", bufs=1) as layp,
            tc.tile_pool(name="fin", bufs=3) as finp,
            tc.tile_pool(name="psmm", bufs=2, space="PSUM") as psmm,
            tc.tile_pool(name="pswin", bufs=2, space="PSUM") as pswin,
            tc.tile_pool(name="psad", bufs=2, space="PSUM") as psad,
            tc.tile_pool(name="pstr", bufs=1, space="PSUM") as pstr,
            tc.tile_pool(name="pspool", bufs=1, space="PSUM") as pspool,
        ):
            # constants
            iota_f = constp.tile([128, 128], F32)
            nc.gpsimd.iota(iota_f[:], pattern=[[1, 128]], base=0,
                           channel_multiplier=0, allow_small_or_imprecise_dtypes=True)
            iota_b = constp.tile([128, 128], BF16)
            nc.vector.tensor_copy(out=iota_b[:], in_=iota_f[:])
            ident = constp.tile([128, 128], BF16)
            make_identity(nc, ident[:])

            waug_sb, bias_sb = [], []
            for li, L in enumerate(cfg.layers):
                cks = []
                for k in range(0, L["d_in"], 128):
                    kc = min(128, L["d_in"] - k)
                    wt = wtsp.tile([kc, L["ROWP"]], BF16, tag=f"w{li}_{k}")
                    nc.sync.dma_start(out=wt[:], in_=waug_p[li][k:k + kc, :])
                    cks.append(wt)
                waug_sb.append(cks)
                bt = wtsp.tile([128, L["db"]], F32, tag=f"b{li}")
                nc.sync.dma_start(out=bt[:], in_=bias_p[li][:, :])
                bias_sb.append(bt)

            pool_ps = pspool.tile([B, cfg.layers[-1]["C"] + 1], F32)
            esrcL_sb = wtsp.tile([128, TOTLO * 8], I16, tag="esrcL")
            nc.sync.dma_start(out=esrcL_sb[:], in_=esrcL_p[:, :])
            esrcH_sb = wtsp.tile([128, TOTHI * 8], I16, tag="esrcH")
            nc.sync.dma_start(out=esrcH_sb[:], in_=esrcH_p[:, :])
            batchw_sb = wtsp.tile([128, NW], BF16, tag="batchwsb")
            nc.sync.dma_start(out=batchw_sb[:], in_=batchw_p[:, :])

            # ---------------- layer-0 projection prologue (local table)
            L0 = cfg.layers[0]
            for mw in range(NW):
                ps = psmm.tile([128, L0["ROWP"]], F32, tag="ps")
                lh = projp.tile([128, 128], BF16, tag="lh")
                nc.sync.dma_start(out=lh[:], in_=xT_p[:, mw * 128:(mw + 1) * 128])
                nc.tensor.matmul(out=ps[:], lhsT=lh[:], rhs=waug_sb[0][0][:],
                                 start=True, stop=True)
                tabt = projp.tile([128, L0["ROWP"]], BF16, tag="tabt")
                nc.scalar.activation(out=tabt[:], in_=ps[:], func=Copy)
                nc.sync.dma_start(out=tabloc[0][mw * 128:(mw + 1) * 128, :], in_=tabt[:])

            for li, L in enumerate(cfg.layers):
                d_in, d_out, H, C = L["d_in"], L["d_out"], L["H"], L["C"]
                R2, ROWP, concat, db = L["R2"], L["ROWP"], L["concat"], L["db"]

                LA = layp.tile([128, NW, d_out], BF16, tag="la")
                if li == NL - 1:
                    bsel = layp.tile([128, NW, B], BF16, tag="bsel")
                    nc.vector.tensor_tensor(
                        out=bsel[:],
                        in0=batchw_sb[:].unsqueeze(2).to_broadcast([128, NW, B]),
                        in1=iota_b[:, :B].unsqueeze(1).to_broadcast([128, NW, B]),
                        op=mybir.AluOpType.is_equal,
                    )

                # ---- half-layer tail: bias+GELU (+head mean), project or pool
                def emit_tail(gi, li=li, L=L, LA=LA, concat=concat, d_out=d_out,
                              H=H, C=C, db=db):
                    h0, h1 = (0, NWH) if gi == 0 else (NWH, NW)
                    nwh = h1 - h0
                    if concat:
                        nc.vector.tensor_add(
                            out=LA[:, h0:h1, :], in0=LA[:, h0:h1, :],
                            in1=bias_sb[li][:].unsqueeze(1).to_broadcast([128, nwh, d_out]))
                        HN = layp.tile([128, nwh, db], BF16, tag="hn")
                        nc.scalar.activation(out=HN[:], in_=LA[:, h0:h1, :], func=Gelu)
                    else:
                        LA4 = LA[:, h0:h1, :].rearrange("p w (h c) -> p w h c", h=H)
                        hm = layp.tile([128, nwh, C], BF16, tag="hm")
                        nc.vector.tensor_add(out=hm[:], in0=LA4[:, :, 0, :], in1=LA4[:, :, 1, :])
                        nc.vector.tensor_add(out=hm[:], in0=hm[:], in1=LA4[:, :, 2, :])
                        nc.vector.tensor_add(out=hm[:], in0=hm[:], in1=LA4[:, :, 3, :])
                        hb = layp.tile([128, nwh, C], BF16, tag="hb")
                        nc.vector.tensor_scalar_mul(hb[:], hm[:], 1.0 / H)
                        nc.vector.tensor_add(
                            out=hb[:], in0=hb[:],
                            in1=bias_sb[li][:].unsqueeze(1).to_broadcast([128, nwh, C]))
                        HN = layp.tile([128, nwh, C + 1], BF16, tag="hn")
                        nc.scalar.activation(out=HN[:, :, :C], in_=hb[:], func=Gelu)
                        nc.vector.memset(HN[:, :, C:], 1.0)

                    if li < NL - 1:
                        Ln = cfg.layers[li + 1]
                        nk = (db + 127) // 128
                        for w in range(h0, h1):
                            ps2 = psmm.tile([128, Ln["ROWP"]], F32, tag="ps")
                            for ki, k in enumerate(range(0, db, 128)):
                                kc = min(128, db - k)
                                pt = pstr.tile([kc, 128], BF16, tag="pt")
                                nc.tensor.transpose(out=pt[:], in_=HN[:, w - h0, k:k + kc],
                                                    identity=ident[:])
                                ht = finp.tile([kc, 128], BF16, tag="ht")
                                nc.scalar.activation(out=ht[:], in_=pt[:], func=Copy)
                                nc.tensor.matmul(out=ps2[:], lhsT=ht[:],
                                                 rhs=waug_sb[li + 1][ki][:],
                                                 start=(ki == 0), stop=(ki == nk - 1))
                            tabt2 = projp.tile([128, Ln["ROWP"]], BF16, tag="tabt")
                            nc.scalar.activation(out=tabt2[:], in_=ps2[:], func=Copy)
                            nc.sync.dma_start(out=tabloc[li + 1][w * 128:(w + 1) * 128, :],
                                              in_=tabt2[:])
                        # half-table done: AllGather it now, overlapping the
                        # rest of this layer
                        if gi == 0:
                            nc.gpsimd.collective_compute(
                                "AllGather", mybir.AluOpType.bypass, replica_groups=rg,
                                ins=[tabloc[li + 1][0:HNPA, :]],
                                outs=[tabfA[li + 1][:, :]])
                        else:
                            nc.gpsimd.collective_compute(
                                "AllGather", mybir.AluOpType.bypass, replica_groups=rg,
                                ins=[tabloc[li + 1][HNPA:, :]],
                                outs=[tabfB[li + 1][:, :]])
                    else:
                        for w in range(h0, h1):
                            nc.tensor.matmul(out=pool_ps[:], lhsT=bsel[:, w, :],
                                             rhs=HN[:, w - h0, :],
                                             start=(w == 0), stop=(w == NW - 1))

                # ---- rolling gather tiles: lo-gathers run PF chunks ahead
                Glo_t, Ghi_t = {}, {}

                def lo_gather(c2, li=li, ROWP=ROWP):
                    inf2 = cinfo[c2]
                    n2 = inf2["nlo"]
                    g = glop.tile([128, max(n2, 1), ROWP], BF16, tag="Glo")
                    Glo_t[c2] = g
                    if n2:
                        nc.gpsimd.dma_gather(
                            g[:, 0:n2, :], tabfA[li][:, :],
                            esrcL_sb[:, inf2["lo_base"] * 8:(inf2["lo_base"] + n2) * 8],
                            n2 * 128, n2 * 128, ROWP, elem_step=ROWP,
                            single_packet=False)

                def hi_gather(c2, li=li, ROWP=ROWP):
                    inf2 = cinfo[c2]
                    n2 = inf2["nhi"]
                    g = ghip.tile([128, max(n2, 1), ROWP], BF16, tag="Ghi")
                    Ghi_t[c2] = g
                    if n2:
                        nc.gpsimd.dma_gather(
                            g[:, 0:n2, :], tabfB[li][:, :],
                            esrcH_sb[:, inf2["hi_base"] * 8:(inf2["hi_base"] + n2) * 8],
                            n2 * 128, n2 * 128, ROWP, elem_step=ROWP,
                            single_packet=False)

                if li > 0:
                    for k in range(min(PF, len(chunks))):
                        lo_gather(k)

                for ci, (c0, c1) in enumerate(chunks):
                    nw_c = c1 - c0
                    inf = cinfo[ci]
                    tile0, nlo, nhi = inf["tile0"], inf["nlo"], inf["nhi"]
                    ntc = nlo + nhi
                    o0 = tile0

                    if li == 0:
                        G0 = glop.tile([128, ntc, ROWP], BF16, tag="Glo")
                        xs = ghip.tile([128, ntc, 128], BF16, tag="Ghi")
                        nc.sync.dma_start(out=xs[:], in_=xE_p[:, o0:o0 + ntc, :])
                        for j0 in range(0, ntc, 4):
                            j1 = min(j0 + 4, ntc)
                            pg = psad.tile([128, (j1 - j0) * ROWP], F32, tag="pa")
                            for k, j in enumerate(range(j0, j1)):
                                nc.tensor.matmul(out=pg[:, k * ROWP:(k + 1) * ROWP],
                                                 lhsT=xs[:, j, :],
                                                 rhs=waug_sb[0][0][:],
                                                 start=True, stop=True)
                            nc.scalar.activation(
                                out=G0[:, j0:j1, :],
                                in_=pg[:].rearrange("p (t r) -> p t r", r=ROWP),
                                func=Copy)
                        Glo, Ghi = G0, G0
                        hoff = 0
                    else:
                        hi_gather(ci)
                        if ci + PF < len(chunks):
                            lo_gather(ci + PF)
                        Glo = Glo_t.pop(ci)
                        Ghi = Ghi_t.pop(ci)
                        hoff = nlo

                    def gpart(ja, jb):
                        """source-row tile range -> (tile, local ja)"""
                        if ja < nlo or hoff == 0:
                            return Glo, ja
                        return Ghi, ja - hoff

                    S = edgep.tile([128, ntc, 128], BF16, tag="S")
                    nc.sync.dma_start(out=S[:], in_=S_p[:, o0:o0 + ntc, :])
                    sd = edgep.tile([128, ntc, 128], BF16, tag="sd")
                    nc.sync.dma_start(out=sd[:], in_=sd_p[:, o0:o0 + ntc, :])

                    xl = edgep.tile([128, nw_c, ROWP], BF16, tag="xl")
                    for wi in range(nw_c):
                        w = c0 + wi
                        nc.sync.dma_start(
                            out=xl[:, wi, :],
                            in_=tabloc[li][w * 128:(w + 1) * 128, :])

                    # per-edge a_dst via one-hot matmuls; z = a_src + a_dst
                    z = edgep.tile([128, ntc, H], BF16, tag="z")
                    for wi in range(nw_c):
                        w = c0 + wi
                        ntw = int(twl[w] + twh[w])
                        pa = psad.tile([128, ntw * H], F32, tag="pa")
                        idx = 0
                        wranges = []
                        if twl[w]:
                            wranges.append((lo_t0[w] - o0, lo_t0[w] - o0 + int(twl[w])))
                        if twh[w]:
                            wranges.append((hi_t0[w] - o0, hi_t0[w] - o0 + int(twh[w])))
                        for (ja, jb) in wranges:
                            for j in range(ja, jb):
                                nc.tensor.matmul(
                                    out=pa[:, idx * H:(idx + 1) * H],
                                    lhsT=sd[:, j, :],
                                    rhs=xl[:, wi, d_out + H:d_out + 2 * H],
                                    start=True, stop=True)
                                idx += 1
                        pa4 = pa[:].rearrange("p (t h) -> p t h", h=H)
                        idx = 0
                        for (ja, jb) in wranges:
                            gt, ga = gpart(ja, jb)
                            nc.vector.tensor_add(
                                out=z[:, ja:jb, :],
                                in0=gt[:, ga:ga + (jb - ja), d_out:d_out + H],
                                in1=pa4[:, idx:idx + (jb - ja), :])
                            idx += jb - ja

                    # p = exp(leaky_relu(z))
                    zs = edgep.tile([128, ntc, H], BF16, tag="zs")
                    nc.scalar.activation(out=zs[:], in_=z[:], func=Copy, scale=0.2)
                    zm = edgep.tile([128, ntc, H], BF16, tag="zm")
                    nc.vector.tensor_max(out=zm[:], in0=z[:], in1=zs[:])
                    pf = edgep.tile([128, ntc, H], BF16, tag="pf")
                    nc.scalar.activation(out=pf[:], in_=zm[:], func=Exp)

                    # MT[e, :d_out] = p[e,h] * xp[src_e, h, :]; MT[e, d_out+h] = p[e,h]
                    MT = edgep.tile([128, ntc, R2], BF16, tag="MT")
                    nc.scalar.activation(out=MT[:, :, d_out:], in_=pf[:], func=Copy)
                    if li == 0:
                        nc.vector.tensor_mul(
                            out=MT[:, :, 0:d_out].rearrange("p t (h c) -> p t h c", h=H),
                            in0=G0[:, :, 0:d_out].rearrange("p t (h c) -> p t h c", h=H),
                            in1=pf[:].unsqueeze(3).to_broadcast([128, ntc, H, C]),
                        )
                    else:
                        if nlo:
                            nc.vector.tensor_mul(
                                out=MT[:, 0:nlo, 0:d_out].rearrange("p t (h c) -> p t h c", h=H),
                                in0=Glo[:, 0:nlo, 0:d_out].rearrange("p t (h c) -> p t h c", h=H),
                                in1=pf[:, 0:nlo].unsqueeze(3).to_broadcast([128, nlo, H, C]),
                            )
                        if nhi:
                            nc.vector.tensor_mul(
                                out=MT[:, nlo:ntc, 0:d_out].rearrange("p t (h c) -> p t h c", h=H),
                                in0=Ghi[:, 0:nhi, 0:d_out].rearrange("p t (h c) -> p t h c", h=H),
                                in1=pf[:, nlo:ntc].unsqueeze(3).to_broadcast([128, nhi, H, C]),
                            )

                    # self-loop terms as one extra (identity) matmul per window
                    zl = edgep.tile([128, nw_c, H], BF16, tag="zl")
                    nc.vector.tensor_add(out=zl[:], in0=xl[:, :, d_out:d_out + H],
                                         in1=xl[:, :, d_out + H:d_out + 2 * H])
                    zl2 = edgep.tile([128, nw_c, H], BF16, tag="zl2")
                    nc.scalar.activation(out=zl2[:], in_=zl[:], func=Copy, scale=0.2)
                    zlm = edgep.tile([128, nw_c, H], BF16, tag="zlm")
                    nc.vector.tensor_max(out=zlm[:], in0=zl[:], in1=zl2[:])
                    pl = edgep.tile([128, nw_c, H], BF16, tag="pl")
                    nc.scalar.activation(out=pl[:], in_=zlm[:], func=Exp)
                    MTs = edgep.tile([128, nw_c, R2], BF16, tag="MTs")
                    nc.scalar.activation(out=MTs[:, :, d_out:], in_=pl[:], func=Copy)
                    nc.vector.tensor_mul(
                        out=MTs[:, :, 0:d_out].rearrange("p t (h c) -> p t h c", h=H),
                        in0=xl[:, :, 0:d_out].rearrange("p t (h c) -> p t h c", h=H),
                        in1=pl[:].unsqueeze(3).to_broadcast([128, nw_c, H, C]),
                    )

                    for wi in range(nw_c):
                        w = c0 + wi
                        ranges = []
                        if twl[w]:
                            ranges.append((lo_t0[w] - o0, lo_t0[w] - o0 + int(twl[w])))
                        if twh[w]:
                            ranges.append((hi_t0[w] - o0, hi_t0[w] - o0 + int(twh[w])))
                        jfirst = ranges[0][0]
                        ps_w = pswin.tile([128, R2], F32)
                        for (ja, jb) in ranges:
                            for j in range(ja, jb):
                                nc.tensor.matmul(out=ps_w[:], lhsT=S[:, j, :],
                                                 rhs=MT[:, j, :],
                                                 start=(j == jfirst), stop=False)
                        nc.tensor.matmul(out=ps_w[:], lhsT=ident[:],
                                         rhs=MTs[:, wi, :], start=False, stop=True)
                        rcp = finp.tile([128, H], F32, tag="rcp")
                        nc.vector.reciprocal(out=rcp[:], in_=ps_w[:, d_out:])
                        for h in range(H):
                            nc.scalar.activation(
                                out=LA[:, w, h * C:(h + 1) * C],
                                in_=ps_w[:, h * C:(h + 1) * C],
                                func=Copy, scale=rcp[:, h:h + 1])

                    if ci == NCH_A - 1:
                        emit_tail(0)

                emit_tail(1)

            # ---------------- final pooling: AllReduce partials, divide
            C = cfg.layers[-1]["C"]
            pps = finp.tile([B, C + 1], F32, tag="pps")
            nc.vector.tensor_copy(out=pps[:], in_=pool_ps[:])
            nc.sync.dma_start(out=poolpart[:, :], in_=pps[:])
            nc.gpsimd.collective_compute(
                "AllReduce", mybir.AluOpType.add, replica_groups=rg,
                ins=[poolpart[:, :]], outs=[poolsum[:, :]],
            )
            pl2 = finp.tile([B, C + 1], F32, tag="pl2")
            nc.sync.dma_start(out=pl2[:], in_=poolsum[:, :])
            cnt = finp.tile([B, 1], F32, tag="cnt")
            nc.vector.tensor_scalar_max(cnt[:], pl2[:, C:C + 1], 1.0)
            rc = finp.tile([B, 1], F32, tag="rc")
            nc.vector.reciprocal(out=rc[:], in_=cnt[:])
            om = finp.tile([B, C], F32, tag="om")
            nc.vector.tensor_mul(out=om[:], in0=pl2[:, :C],
                                 in1=rc[:, :1].to_broadcast([B, C]))
            nc.sync.dma_start(out=out_p[:, :], in_=om[:])

    nc.finalize()
    return nc


# ---------------------------------------------------------------- entry
def _prep_and_build(cfg, x, edge_index, batch, Ws, As, Ad, Bs):
    in_maps, meta = _host_prep(cfg, np.asarray(x), np.asarray(edge_index),
                               np.asarray(batch), Ws, As, Ad, Bs)
    nc = _build_program(cfg, meta)
    return nc, in_maps


def kernel(x, edge_index, batch, W0, as0, ad0, b0, W1, as1, ad1, b1, W2, as2, ad2, b2):
    from concourse.bass_utils import run_bass_kernel_spmd

    cfg = REAL_CFG
    nc, in_maps = _prep_and_build(
        cfg, x, edge_index, batch,
        [np.asarray(W0), np.asarray(W1), np.asarray(W2)],
        [np.asarray(as0), np.asarray(as1), np.asarray(as2)],
        [np.asarray(ad0), np.asarray(ad1), np.asarray(ad2)],
        [np.asarray(b0), np.asarray(b1), np.asarray(b2)],
    )
    res = run_bass_kernel_spmd(nc, in_maps, list(range(cfg.NC)))
    return np.asarray(res.results[0]["out"], dtype=np.float32)
